# revision 20
# baseline (speedup 1.0000x reference)
"""GRAM-style GNN message passing kernel for 8 Trainium2 NeuronCores.

Model (see reference):
  1. Ontology attention: gather leaf/ancestor embedding rows, small MLP
     (tanh) -> softmax over L=5 ancestors -> emb [V, E] (weighted sum of
     ancestor embeddings).
  2. x_emb = tanh(x @ emb)          [T, B, E]   (the big GEMM)
  3. 50-step GRU scan over T        [T, B, H]
  4. out = softmax(hidden @ W_out + b_out) * mask

Sharding: phase 1 sharded over V (AllGather assembles [V, E] on every
core), phases 2-4 data-parallel over batch B (16 per core).

v2 layout:
  - x^T is prefetched at t=0 into one SBUF slab via a few big DMAs
    (the per-tile DMA issue cost on the Sync engine dominated v1).
  - Phase B runs 126 back-to-back bf16 matmuls from SBUF (PE ramps to
    full p-state), then the GRU input-gates Gx = wih.T @ x_emb are
    precomputed for all T with biases folded (z negated so both r and
    1-z come out of one Sigmoid).
  - GRU loop does only 3 small bf16 matmuls per step, accumulating onto
    a PSUM tile prefilled by the vector engine with Gx, and h is kept
    in bf16.
  - Output softmax runs as a tail after the scan: Exp and Sigmoid live
    in different ACT tables, so interleaving them reloads a 1.3us table
    every 8 steps.
"""

import numpy as np
import ml_dtypes

import concourse.bass as bass
import concourse.mybir as mybir
import concourse.tile as tile
from concourse import bacc
from concourse.bass_utils import run_bass_kernel_spmd

FP32 = mybir.dt.float32
FP32R = mybir.dt.float32r
BF16 = mybir.dt.bfloat16

AF = mybir.ActivationFunctionType
ALU = mybir.AluOpType
AX = mybir.AxisListType

T, B, V = 50, 128, 8000
A = 728
NEMB = V + A          # 8728
E, H, ATT, C, L = 128, 128, 100, 283, 5
NCORES = 8
BL = B // NCORES      # 16 batch per core
M = T * BL            # 800 tokens per core (m = t*BL + b)
VS = V // NCORES      # 1000 v rows per core for phase 1
VP = 1024             # padded v-shard
NIDX = L * VP         # 5120 gather indices per table
NVT = VP // 128       # 8 v-tiles per shard
KT = (V + 127) // 128 # 63 contraction tiles for the big GEMM
MT = (M + 127) // 128 # 7 m-tiles for the output phase
NH = M // 2           # 400: half of M (psum bank limit is 512 fp32)

_CACHE = {}


def _build_nc(debug=False):
    nc = bacc.Bacc(
        "TRN2",
        target_bir_lowering=False,
        debug=False,
        num_devices=NCORES,
    )

    # ---- DRAM I/O ----
    xslab_d = nc.dram_tensor("xslab_d", [128, KT * M], BF16, kind="ExternalInput").ap()
    leT_d = nc.dram_tensor("leT_d", [E, NIDX], BF16, kind="ExternalInput").ap()
    anT_d = nc.dram_tensor("anT_d", [E, NIDX], BF16, kind="ExternalInput").ap()
    an3_d = nc.dram_tensor("an3_d", [128, NIDX], BF16, kind="ExternalInput").ap()
    watt_top = nc.dram_tensor("watt_top", [E, ATT], BF16, kind="ExternalInput").ap()
    watt_bot = nc.dram_tensor("watt_bot", [E, ATT], BF16, kind="ExternalInput").ap()
    vatt = nc.dram_tensor("vatt", [ATT, 1], BF16, kind="ExternalInput").ap()
    batt = nc.dram_tensor("batt", [ATT, 1], FP32, kind="ExternalInput").ap()
    # wih^T [E, 3H] fp32 (z block negated), for the fp32r Gx matmuls
    wihr = nc.dram_tensor("wihr", [E, 3 * H], FP32R, kind="ExternalInput").ap()
    # whh^T [H, 3H] bf16 (z block negated), for the in-loop matmuls
    whhb = nc.dram_tensor("whhb", [H, 3 * H], BF16, kind="ExternalInput").ap()
    b_r = nc.dram_tensor("b_r", [H, 1], FP32, kind="ExternalInput").ap()
    b_z_neg = nc.dram_tensor("b_z_neg", [H, 1], FP32, kind="ExternalInput").ap()
    b_in = nc.dram_tensor("b_in", [H, 1], FP32, kind="ExternalInput").ap()
    b_hn = nc.dram_tensor("b_hn", [H, 1], FP32, kind="ExternalInput").ap()
    wout = nc.dram_tensor("wout", [H, C], BF16, kind="ExternalInput").ap()
    bout = nc.dram_tensor("bout", [1, C], FP32, kind="ExternalInput").ap()
    maskcol = nc.dram_tensor("maskcol", [M, 1], FP32, kind="ExternalInput").ap()
    out_d = nc.dram_tensor("out", [M, C], FP32, kind="ExternalOutput").ap()

    emb_shard = nc.dram_tensor("emb_shard", [VS, E], BF16).ap()
    emb_full = nc.dram_tensor("emb_full", [V, E], BF16, addr_space="Shared").ap()

    if debug:
        dbg_xemb = nc.dram_tensor("dbg_xemb", [E, M], FP32R, kind="ExternalOutput").ap()
        dbg_hid = nc.dram_tensor("dbg_hid", [H, M], FP32, kind="ExternalOutput").ap()
    else:
        dbg_xemb = dbg_hid = None

    with tile.TileContext(nc) as tc:
        _emit(nc, tc, locals())
    nc.compile()
    return nc


def _emit(nc, tc, t):
    xslab_d = t["xslab_d"]
    leT_d, anT_d, an3_d = t["leT_d"], t["anT_d"], t["an3_d"]
    watt_top, watt_bot, vatt, batt = t["watt_top"], t["watt_bot"], t["vatt"], t["batt"]
    wihr, whhb = t["wihr"], t["whhb"]
    b_r, b_z_neg, b_in, b_hn = t["b_r"], t["b_z_neg"], t["b_in"], t["b_hn"]
    wout, bout, maskcol, out_d = t["wout"], t["bout"], t["maskcol"], t["out_d"]
    emb_shard, emb_full = t["emb_shard"], t["emb_full"]
    dbg_xemb, dbg_hid = t["dbg_xemb"], t["dbg_hid"]

    with tc.tile_pool(name="const", bufs=1) as constp:
        # ---- DMA priority: the MLP weights, then the gather tables (they
        # gate phase A) split across all three queues; the big x^T slab last.
        watt_top_sb = constp.tile([E, ATT], BF16)
        nc.scalar.dma_start(watt_top_sb[:], watt_top[:, :])
        watt_bot_sb = constp.tile([E, ATT], BF16)
        nc.scalar.dma_start(watt_bot_sb[:], watt_bot[:, :])
        vatt_sb = constp.tile([ATT, 1], BF16)
        nc.scalar.dma_start(vatt_sb[:], vatt[:, :])
        batt_sb = constp.tile([ATT, 1], FP32)
        nc.scalar.dma_start(batt_sb[:], batt[:, :])

        gathp = tc.alloc_tile_pool(name="gath", bufs=1)
        leT = gathp.tile([128, NIDX], BF16)   # emb rows as columns
        anT = gathp.tile([128, NIDX], BF16)
        an3 = gathp.tile([128, NIDX], BF16)   # emb rows as rows, per (l, vt)
        NQ = NIDX // 4
        for q, eng in enumerate([nc.scalar, nc.sync, nc.gpsimd, nc.scalar]):
            eng.dma_start(leT[:, q * NQ : (q + 1) * NQ], leT_d[:, q * NQ : (q + 1) * NQ])
        for q, eng in enumerate([nc.sync, nc.gpsimd, nc.scalar, nc.sync]):
            eng.dma_start(anT[:, q * NQ : (q + 1) * NQ], anT_d[:, q * NQ : (q + 1) * NQ])
        for q, eng in enumerate([nc.gpsimd, nc.scalar, nc.sync, nc.gpsimd]):
            eng.dma_start(an3[:, q * NQ : (q + 1) * NQ], an3_d[:, q * NQ : (q + 1) * NQ])

        # ---- small constants ----
        wihr_sb = constp.tile([E, 3 * H], FP32R)
        nc.scalar.dma_start(wihr_sb[:], wihr[:, :])
        whhb_sb = constp.tile([H, 3 * H], BF16)
        nc.scalar.dma_start(whhb_sb[:], whhb[:, :])
        b_r_sb = constp.tile([H, 1], FP32)
        nc.scalar.dma_start(b_r_sb[:], b_r[:, :])
        b_zn_sb = constp.tile([H, 1], FP32)
        nc.scalar.dma_start(b_zn_sb[:], b_z_neg[:, :])
        b_in_sb = constp.tile([H, 1], FP32)
        nc.scalar.dma_start(b_in_sb[:], b_in[:, :])
        b_hn_sb = constp.tile([H, 1], FP32)
        nc.scalar.dma_start(b_hn_sb[:], b_hn[:, :])
        wout_sb = constp.tile([H, C], BF16)
        nc.scalar.dma_start(wout_sb[:], wout[:, :])
        bout_sb = constp.tile([1, C], FP32)
        nc.scalar.dma_start(bout_sb[:], bout[:, :])
        # mask columns per m-tile: masks_sb[p, k] = mask[k*128 + p]
        masks_sb = constp.tile([128, MT], FP32)
        nfull = (M // 128) * 128
        nc.scalar.dma_start(
            masks_sb[:, 0 : M // 128],
            maskcol[0:nfull, 0:1].rearrange("(a p) o -> p (a o)", p=128),
        )
        if M % 128:
            nc.scalar.dma_start(
                masks_sb[0 : M % 128, M // 128 : M // 128 + 1],
                maskcol[nfull:M, 0:1],
            )

        # ---- x^T slab prefetch: big DMAs, queued after the tables ----
        xslab = constp.tile([128, KT * M], BF16)
        bounds = [0, 16, 32, 48, KT]
        for i, eng in enumerate([nc.sync, nc.gpsimd, nc.sync, nc.gpsimd]):
            c0, c1 = bounds[i] * M, bounds[i + 1] * M
            eng.dma_start(xslab[:, c0:c1], xslab_d[:, c0:c1])

        # =====================================================================
        # Phase A: ontology attention on the local v-shard -> emb_shard
        # pre values are bounded (|pre| <= sum|v_att| ~ 50, tanh'd mlp), so
        # exp never overflows fp32 and the max-subtraction is skipped.
        # =====================================================================
        with (
            tc.tile_pool(name="pa_sb", bufs=2) as pa_sb,
            tc.tile_pool(name="pa_ps", bufs=2, space="PSUM") as pa_ps,
            tc.tile_pool(name="pre_ps", bufs=1, space="PSUM") as pre_ps,
        ):
            an3v = an3[:].rearrange("p (i e) -> p i e", e=E)

            # MLP: mlp_T[a, j] = tanh(watt_top.T @ leT + watt_bot.T @ anT + b)
            # pre-matmuls for (l, vt) interleave as soon as their chunk lands.
            mlp_sb = gathp.tile([ATT, NIDX], BF16)
            psp = pre_ps.tile([128, NVT * L], FP32)  # col = vt*L + l
            CH = 512
            NCH = NIDX // CH
            for ch in range(NCH + 1):
                if ch < NCH:
                    ps = pa_ps.tile([ATT, CH], FP32, tag="mlp")
                    sl = bass.ts(ch, CH)
                    nc.tensor.matmul(ps[:], watt_top_sb[:], leT[:, sl], start=True, stop=False)
                    nc.tensor.matmul(ps[:], watt_bot_sb[:], anT[:, sl], start=False, stop=True)
                    nc.scalar.activation(mlp_sb[:, sl], ps[:], AF.Tanh, bias=batt_sb[:, 0:1])
                # pre-matmuls lag one chunk so the tensor engine never
                # stalls waiting for the tanh of the chunk just issued
                if ch > 0:
                    for j in range((ch - 1) * 4, ch * 4):
                        l, vt = j // NVT, j % NVT
                        nc.tensor.matmul(
                            psp[:, vt * L + l : vt * L + l + 1],
                            mlp_sb[:, l * VP + vt * 128 : l * VP + (vt + 1) * 128],
                            vatt_sb[:],
                            start=True,
                            stop=True,
                            skip_group_check=True,
                        )

            # batched softmax over L for all vt (no max subtraction)
            att = pa_sb.tile([128, NVT * L], FP32, tag="att")
            nc.scalar.activation(att[:], psp[:], AF.Exp)
            att3 = att[:].rearrange("p (vt l) -> p vt l", l=L)
            asum = pa_sb.tile([128, NVT], FP32, tag="asum")
            nc.vector.tensor_reduce(asum[:], att3, AX.X, ALU.add)
            arec = pa_sb.tile([128, NVT], FP32, tag="arec")
            nc.vector.reciprocal(arec[:], asum[:])

            # weighted ancestor sum per vt
            for vt in range(NVT):
                acc = pa_sb.tile([128, E], FP32, tag="acc")
                nc.vector.tensor_scalar(
                    acc[:], an3v[:, 0 * NVT + vt, :], att[:, vt * L : vt * L + 1],
                    None, op0=ALU.mult,
                )
                for l in range(1, L):
                    nc.vector.scalar_tensor_tensor(
                        acc[:],
                        an3v[:, l * NVT + vt, :],
                        att[:, vt * L + l : vt * L + l + 1],
                        acc[:],
                        op0=ALU.mult,
                        op1=ALU.add,
                    )
                embt = pa_sb.tile([128, E], BF16, tag="embt")
                nc.vector.tensor_scalar(
                    embt[:], acc[:], arec[:, vt : vt + 1], None, op0=ALU.mult
                )
                rows = min(128, VS - vt * 128)
                if rows > 0:
                    nc.scalar.dma_start(
                        emb_shard[vt * 128 : vt * 128 + rows, :], embt[0:rows, :]
                    )
        gathp.release()

        # AllGather the embedding table across the 8 cores.
        nc.gpsimd.collective_compute(
            "AllGather",
            ALU.bypass,
            replica_groups=[list(range(NCORES))],
            ins=[emb_shard[:, :]],
            outs=[emb_full[:, :]],
        )

        # emb slab: [128, KT*128], block kt holds emb rows kt*128..+128.
        # Split into 8 DMAs so the first phase B matmuls start early.
        embslab = constp.tile([128, KT * 128], BF16)
        engs = [nc.sync, nc.gpsimd, nc.scalar]
        for pc in range(8):
            a0, a1 = pc * 8, min((pc + 1) * 8, KT - 1)
            engs[pc % 3].dma_start(
                embslab[:, a0 * 128 : a1 * 128].rearrange("p (a e) -> p a e", e=E),
                emb_full[a0 * 128 : a1 * 128, :].rearrange("(a p) e -> p a e", p=128),
            )
        nfull_v = (KT - 1) * 128  # 7936
        nc.gpsimd.dma_start(
            embslab[0 : V - nfull_v, nfull_v : nfull_v + 128],
            emb_full[nfull_v:V, :],
        )

        # =====================================================================
        # Phase B: x_emb^T = tanh(emb^T @ x^T)   [E, M]
        # =====================================================================
        xemb = constp.tile([E, M], FP32R)
        with tc.tile_pool(name="pb_ps", bufs=1, space="PSUM") as pb_ps:
            ps_a = pb_ps.tile([128, NH], FP32, tag="ps_a")
            ps_b = pb_ps.tile([128, NH], FP32, tag="ps_b")
            for kt in range(KT):
                kp = min(128, V - kt * 128)
                lhs = embslab[0:kp, kt * 128 : kt * 128 + E]
                st, sp = kt == 0, kt == KT - 1
                nc.tensor.matmul(
                    ps_a[:], lhs, xslab[0:kp, kt * M : kt * M + NH], start=st, stop=sp
                )
                nc.tensor.matmul(
                    ps_b[:], lhs, xslab[0:kp, kt * M + NH : kt * M + M], start=st, stop=sp
                )
            nc.scalar.activation(xemb[:, 0:NH], ps_a[:], AF.Tanh)
            nc.scalar.activation(xemb[:, NH:M], ps_b[:], AF.Tanh)
            if dbg_xemb is not None:
                nc.sync.dma_start(dbg_xemb[:, :], xemb[:])

        # =====================================================================
        # Gx precompute: input halves of all gates, biases folded.
        #   GxRZ[h, t*32+0:16]  = wih_r.T @ xemb_t + (b_ih_r + b_hh_r)
        #   GxRZ[h, t*32+16:32] = -(wih_z.T @ xemb_t + b_ih_z + b_hh_z)
        #   Gxn [h, t*16:+16]   = wih_n.T @ xemb_t + b_in
        # (z block of wihr pre-negated host-side, b_zn = -(b_ih_z+b_hh_z))
        # =====================================================================
        # gxall: per step t, 48 columns [gx_r+b_r | -(gx_z+b_z) | b_hn rep]
        gxall = constp.tile([H, T * 3 * BL], FP32)
        gxn = constp.tile([H, M], FP32)
        gx3 = gxall[:].rearrange("p (t q) -> p t q", q=3 * BL)
        nc.vector.memset(gx3[:, :, 2 * BL : 3 * BL], 0.0)
        nc.vector.tensor_scalar(
            gx3[:, :, 2 * BL : 3 * BL], gx3[:, :, 2 * BL : 3 * BL],
            b_hn_sb[:, 0:1], None, op0=ALU.add,
        )
        with tc.tile_pool(name="gx_ps", bufs=2, space="PSUM") as gx_ps:
            for half in range(2):
                xe = xemb[:, half * NH : (half + 1) * NH]
                t0, t1_ = half * (T // 2), (half + 1) * (T // 2)
                for g, bias in enumerate([b_r_sb, b_zn_sb, b_in_sb]):
                    psg = gx_ps.tile([H, NH], FP32, tag=f"gx{g % 2}")
                    nc.tensor.matmul(
                        psg[:], wihr_sb[:, g * H : (g + 1) * H], xe,
                        start=True, stop=True,
                    )
                    src = psg[:].rearrange("p (t b) -> p t b", b=BL)
                    if g == 0:
                        dst = gx3[:, t0:t1_, 0:BL]
                    elif g == 1:
                        dst = gx3[:, t0:t1_, BL : 2 * BL]
                    else:
                        dst = gxn[:, half * NH : (half + 1) * NH].rearrange(
                            "p (t b) -> p t b", b=BL
                        )
                    nc.vector.tensor_scalar(dst, src, bias[:, 0:1], None, op0=ALU.add)

        # =====================================================================
        # GRU scan; out logits computed per completed 128-token tile.
        # h kept in bf16 [H, M] (column t*BL+b).
        # =====================================================================
        hid = constp.tile([H, M], BF16)
        h0 = constp.tile([H, BL], BF16)
        nc.vector.memset(h0[:], 0.0)
        logits = constp.tile([128, MT * C], FP32)
        bb_sb = constp.tile([128, C], FP32)
        ones_sb = constp.tile([1, 128], FP32)
        nc.vector.memset(ones_sb[:], 1.0)

        QW = 3 * BL  # 48 psum columns per step: [r | zc | n]
        with (
            tc.tile_pool(name="gru_ps", bufs=2, space="PSUM") as gru_ps,
            tc.tile_pool(name="o_ps", bufs=2, space="PSUM") as o_ps,
            tc.tile_pool(name="gru_sb", bufs=3) as gru_sb,
        ):
            # broadcast b_out across partitions with a rank-1 matmul
            psbb = o_ps.tile([128, C], FP32, tag="o")
            nc.tensor.matmul(psbb[:], ones_sb[:], bout_sb[:], start=True, stop=True)
            nc.vector.tensor_copy(bb_sb[:], psbb[:])

            # prefill Gx into psum in blocks of 8 steps
            NB = 8
            NBLK = (T + NB - 1) // NB
            g_blk = [None] * NBLK
            g_blk[0] = gru_ps.tile([H, NB * QW], FP32, tag="gblk", name="gblk0")
            nc.vector.tensor_copy(g_blk[0][:], gxall[:, 0 : NB * QW])

            for st_ in range(T):
                blk, j = st_ // NB, st_ % NB
                g = g_blk[blk][:, j * QW : (j + 1) * QW]
                hprev = h0[:] if st_ == 0 else hid[:, (st_ - 1) * BL : st_ * BL]

                # hh matmuls accumulate onto the DVE-prefilled psum tile
                nc.tensor.matmul(
                    g[:, 0:BL], whhb_sb[:, 0:H], hprev,
                    start=False, stop=True, skip_group_check=True,
                )
                nc.tensor.matmul(
                    g[:, BL : 2 * BL], whhb_sb[:, H : 2 * H], hprev,
                    start=False, stop=True, skip_group_check=True,
                )
                nc.tensor.matmul(
                    g[:, 2 * BL : 3 * BL], whhb_sb[:, 2 * H : 3 * H], hprev,
                    start=False, stop=True, skip_group_check=True,
                )

                # r | zc in one sigmoid (z pre-negated => zc = 1-z)
                rz = gru_sb.tile([H, 2 * BL], BF16, tag="rz")
                nc.scalar.activation(rz[:], g[:, 0 : 2 * BL], AF.Sigmoid)

                # n = tanh(gxn + r * (ghn + b_hn))   (b_hn came via prefill)
                t1 = gru_sb.tile([H, BL], FP32, tag="t1")
                nc.vector.tensor_mul(t1[:], g[:, 2 * BL : 3 * BL], rz[:, 0:BL])
                t2 = gru_sb.tile([H, BL], FP32, tag="t2")
                nc.vector.tensor_add(t2[:], t1[:], gxn[:, st_ * BL : (st_ + 1) * BL])

                # mid-block: prefill the next block's psum
                if j == NB // 2 and blk + 1 < NBLK:
                    cols = min(NB * QW, (T - (blk + 1) * NB) * QW)
                    g_blk[blk + 1] = gru_ps.tile(
                        [H, NB * QW], FP32, tag="gblk", name=f"gblk{blk + 1}"
                    )
                    nc.vector.tensor_copy(
                        g_blk[blk + 1][:, 0:cols],
                        gxall[:, (blk + 1) * NB * QW : (blk + 1) * NB * QW + cols],
                    )

                # q = zc*h ; P = h - q run on DVE during the tanh, so only
                # two DVE ops (u, h') remain on the chain after it
                q = gru_sb.tile([H, BL], FP32, tag="q")
                nc.vector.tensor_mul(q[:], rz[:, BL : 2 * BL], hprev)
                P = gru_sb.tile([H, BL], FP32, tag="P")
                nc.vector.tensor_sub(P[:], hprev, q[:])

                n = gru_sb.tile([H, BL], BF16, tag="n")
                nc.scalar.activation(n[:], t2[:], AF.Tanh)

                # h' = zc*n + (h - zc*h)
                u = gru_sb.tile([H, BL], FP32, tag="u")
                nc.vector.tensor_mul(u[:], rz[:, BL : 2 * BL], n[:])
                nc.vector.tensor_add(hid[:, st_ * BL : (st_ + 1) * BL], u[:], P[:])

                # logits for each completed m-tile of 128 tokens
                if (st_ + 1) % 8 == 0 or st_ == T - 1:
                    k = st_ // 8
                    mw = min(128, M - k * 128)
                    pso = o_ps.tile([128, C], FP32, tag="o")
                    nc.tensor.matmul(
                        pso[0:mw, :], hid[:, k * 128 : k * 128 + mw], wout_sb[:],
                        start=True, stop=True,
                    )
                    nc.vector.tensor_add(
                        logits[0:mw, k * C : (k + 1) * C], pso[0:mw, :], bb_sb[0:mw, :]
                    )
                    if dbg_hid is not None:
                        nc.sync.dma_start(
                            dbg_hid[:, k * 128 : k * 128 + mw],
                            hid[:, k * 128 : k * 128 + mw],
                        )

        # =====================================================================
        # Softmax tail (single Exp table load)
        # =====================================================================
        # logits are bounded (|h|<=1, W_out in [0, 0.1]) so exp never
        # overflows fp32 and the max-subtraction is skipped.
        with tc.tile_pool(name="sm_sb", bufs=2) as sm_sb:
            for k in range(MT):
                mw = min(128, M - k * 128)
                lg = logits[0:mw, k * C : (k + 1) * C]
                ex = sm_sb.tile([128, C], FP32, tag="ex")
                ssum = sm_sb.tile([128, 1], FP32, tag="ssum")
                nc.scalar.activation(
                    ex[0:mw, :], lg, AF.Exp, accum_out=ssum[0:mw, 0:1]
                )
                rec = sm_sb.tile([128, 1], FP32, tag="rec")
                nc.vector.reciprocal(rec[0:mw, :], ssum[0:mw, :])
                ob = sm_sb.tile([128, C], FP32, tag="ob")
                nc.vector.tensor_scalar(
                    ob[0:mw, :], ex[0:mw, :], rec[0:mw, 0:1], masks_sb[0:mw, k : k + 1],
                    op0=ALU.mult, op1=ALU.mult,
                )
                eng = [nc.sync, nc.gpsimd, nc.scalar][k % 3]
                eng.dma_start(out_d[k * 128 : k * 128 + mw, :], ob[0:mw, :])


def _prep_inputs(inputs):
    x = np.asarray(inputs["x"], np.float32)
    mask = np.asarray(inputs["mask"], np.float32)
    W_emb = np.asarray(inputs["W_emb"], np.float32)
    W_att = np.asarray(inputs["W_attention"], np.float32)
    b_att = np.asarray(inputs["b_attention"], np.float32)
    v_att = np.asarray(inputs["v_attention"], np.float32)
    w_ih = np.asarray(inputs["gru_w_ih"], np.float32)
    w_hh = np.asarray(inputs["gru_w_hh"], np.float32)
    b_ih = np.asarray(inputs["gru_b_ih"], np.float32)
    b_hh = np.asarray(inputs["gru_b_hh"], np.float32)
    W_out = np.asarray(inputs["W_output"], np.float32)
    b_out = np.asarray(inputs["b_output"], np.float32)
    leaves = np.asarray(inputs["leaves"])
    ancestors = np.asarray(inputs["ancestors"])

    # wih^T [E, 3H] with z block negated (fp32, used via fp32r)
    wih_t = np.ascontiguousarray(w_ih.T).copy()
    wih_t[:, H : 2 * H] = -wih_t[:, H : 2 * H]
    # whh^T [H, 3H] with z block negated (bf16)
    whh_t = np.ascontiguousarray(w_hh.T).copy()
    whh_t[:, H : 2 * H] = -whh_t[:, H : 2 * H]

    shared = {
        "watt_top": np.ascontiguousarray(W_att[:E, :].astype(ml_dtypes.bfloat16)),
        "watt_bot": np.ascontiguousarray(W_att[E:, :].astype(ml_dtypes.bfloat16)),
        "vatt": np.ascontiguousarray(v_att.reshape(ATT, 1).astype(ml_dtypes.bfloat16)),
        "batt": np.ascontiguousarray(b_att.reshape(ATT, 1)),
        "wihr": np.ascontiguousarray(wih_t),
        "whhb": np.ascontiguousarray(whh_t.astype(ml_dtypes.bfloat16)),
        "b_r": np.ascontiguousarray((b_ih[0:H] + b_hh[0:H]).reshape(H, 1)),
        "b_z_neg": np.ascontiguousarray(-(b_ih[H : 2 * H] + b_hh[H : 2 * H]).reshape(H, 1)),
        "b_in": np.ascontiguousarray(b_ih[2 * H : 3 * H].reshape(H, 1)),
        "b_hn": np.ascontiguousarray(b_hh[2 * H : 3 * H].reshape(H, 1)),
        "wout": np.ascontiguousarray(W_out.astype(ml_dtypes.bfloat16)),
        "bout": np.ascontiguousarray(b_out.reshape(1, C)),
    }

    W_bf = W_emb.astype(ml_dtypes.bfloat16)
    in_maps = []
    for c in range(NCORES):
        m = dict(shared)
        xc = x[:, c * BL : (c + 1) * BL, :].reshape(M, V)
        xcT = np.zeros((KT * 128, M), ml_dtypes.bfloat16)
        xcT[:V, :] = np.ascontiguousarray(xc.T).astype(ml_dtypes.bfloat16)
        # slab[p, kt*M + m] = x^T[kt*128 + p, m]
        m["xslab_d"] = np.ascontiguousarray(
            xcT.reshape(KT, 128, M).transpose(1, 0, 2).reshape(128, KT * M)
        )
        m["maskcol"] = np.ascontiguousarray(
            mask[:, c * BL : (c + 1) * BL].reshape(M, 1)
        )
        lv = leaves[c * VS : (c + 1) * VS, :]
        av = ancestors[c * VS : (c + 1) * VS, :]
        le_pad = np.zeros((L, VP), np.int64)   # [l, v] l-major, v padded
        le_pad[:, :VS] = lv.T
        an_pad = np.zeros((L, VP), np.int64)
        an_pad[:, :VS] = av.T
        # pre-gathered, pre-transposed embedding rows (static index prep)
        le_rows = W_bf[le_pad.reshape(-1), :]          # [NIDX, E]
        an_rows = W_bf[an_pad.reshape(-1), :]          # [NIDX, E]
        m["leT_d"] = np.ascontiguousarray(le_rows.T)   # [E, NIDX]
        m["anT_d"] = np.ascontiguousarray(an_rows.T)   # [E, NIDX]
        # an3[p, (l*NVT+vt)*E + e] = an_rows[l*VP + vt*128 + p, e]
        m["an3_d"] = np.ascontiguousarray(
            an_rows.reshape(L, NVT, 128, E).transpose(2, 0, 1, 3).reshape(128, NIDX)
        )
        in_maps.append(m)
    return in_maps


def kernel(**inputs):
    if "nc" not in _CACHE:
        _CACHE["nc"] = _build_nc()
    nc = _CACHE["nc"]
    in_maps = _prep_inputs(inputs)
    res = run_bass_kernel_spmd(nc, in_maps, list(range(NCORES)))
    out = np.empty((T, B, C), np.float32)
    for c in range(NCORES):
        out[:, c * BL : (c + 1) * BL, :] = res.results[c]["out"].reshape(T, BL, C)
    return out


# revision 21
# speedup vs baseline: 1.1681x; 1.1681x over previous
"""GRAM-style GNN message passing kernel for 8 Trainium2 NeuronCores.

Model (see reference):
  1. Ontology attention: gather leaf/ancestor embedding rows, small MLP
     (tanh) -> softmax over L=5 ancestors -> emb [V, E] (weighted sum of
     ancestor embeddings).
  2. x_emb = tanh(x @ emb)          [T, B, E]   (the big GEMM)
  3. 50-step GRU scan over T        [T, B, H]
  4. out = softmax(hidden @ W_out + b_out) * mask

Sharding: phase 1 sharded over V (AllGather assembles [V, E] on every
core), phases 2-4 data-parallel over batch B (16 per core).

v2 layout:
  - x^T is prefetched at t=0 into one SBUF slab via a few big DMAs
    (the per-tile DMA issue cost on the Sync engine dominated v1).
  - Phase B runs 126 back-to-back bf16 matmuls from SBUF (PE ramps to
    full p-state), then the GRU input-gates Gx = wih.T @ x_emb are
    precomputed for all T with biases folded (z negated so both r and
    1-z come out of one Sigmoid).
  - GRU loop does only 3 small bf16 matmuls per step, accumulating onto
    a PSUM tile prefilled by the vector engine with Gx, and h is kept
    in bf16.
  - Output softmax runs as a tail after the scan: Exp and Sigmoid live
    in different ACT tables, so interleaving them reloads a 1.3us table
    every 8 steps.
"""

import numpy as np
import ml_dtypes

import concourse.bass as bass
import concourse.mybir as mybir
import concourse.tile as tile
from concourse import bacc
from concourse.bass_utils import run_bass_kernel_spmd

FP32 = mybir.dt.float32
FP32R = mybir.dt.float32r
BF16 = mybir.dt.bfloat16

AF = mybir.ActivationFunctionType
ALU = mybir.AluOpType
AX = mybir.AxisListType

T, B, V = 50, 128, 8000
A = 728
NEMB = V + A          # 8728
E, H, ATT, C, L = 128, 128, 100, 283, 5
NCORES = 8
BL = B // NCORES      # 16 batch per core
M = T * BL            # 800 tokens per core (m = t*BL + b)
VS = V // NCORES      # 1000 v rows per core for phase 1
VP = 1024             # padded v-shard
NIDX = L * VP         # 5120 gather indices per table
NVT = VP // 128       # 8 v-tiles per shard
KT = (V + 127) // 128 # 63 contraction tiles for the big GEMM
MT = (M + 127) // 128 # 7 m-tiles for the output phase
NH = M // 2           # 400: half of M (psum bank limit is 512 fp32)

_CACHE = {}


def _build_nc(debug=False):
    nc = bacc.Bacc(
        "TRN2",
        target_bir_lowering=False,
        debug=False,
        num_devices=NCORES,
    )

    # ---- DRAM I/O ----
    xslab_d = nc.dram_tensor("xslab_d", [128, KT * M], BF16, kind="ExternalInput").ap()
    leT_d = nc.dram_tensor("leT_d", [E, NIDX], BF16, kind="ExternalInput").ap()
    anT_d = nc.dram_tensor("anT_d", [E, NIDX], BF16, kind="ExternalInput").ap()
    an3_d = nc.dram_tensor("an3_d", [128, NIDX], BF16, kind="ExternalInput").ap()
    watt_top = nc.dram_tensor("watt_top", [E, ATT], BF16, kind="ExternalInput").ap()
    watt_bot = nc.dram_tensor("watt_bot", [E, ATT], BF16, kind="ExternalInput").ap()
    vatt = nc.dram_tensor("vatt", [ATT, 1], BF16, kind="ExternalInput").ap()
    batt = nc.dram_tensor("batt", [ATT, 1], FP32, kind="ExternalInput").ap()
    # wih^T [E, 3H] fp32 (z block negated), for the fp32r Gx matmuls
    wihr = nc.dram_tensor("wihr", [E, 3 * H], FP32R, kind="ExternalInput").ap()
    # whh^T [H, 3H] bf16 (z block negated), for the in-loop matmuls
    whhb = nc.dram_tensor("whhb", [H, 3 * H], BF16, kind="ExternalInput").ap()
    b_r = nc.dram_tensor("b_r", [H, 1], FP32, kind="ExternalInput").ap()
    b_z_neg = nc.dram_tensor("b_z_neg", [H, 1], FP32, kind="ExternalInput").ap()
    b_in = nc.dram_tensor("b_in", [H, 1], FP32, kind="ExternalInput").ap()
    b_hn = nc.dram_tensor("b_hn", [H, 1], FP32, kind="ExternalInput").ap()
    wout = nc.dram_tensor("wout", [H, C], BF16, kind="ExternalInput").ap()
    bout = nc.dram_tensor("bout", [1, C], FP32, kind="ExternalInput").ap()
    maskcol = nc.dram_tensor("maskcol", [M, 1], FP32, kind="ExternalInput").ap()
    out_d = nc.dram_tensor("out", [M, C], FP32, kind="ExternalOutput").ap()

    emb_shard = nc.dram_tensor("emb_shard", [VS, E], BF16).ap()
    emb_full = nc.dram_tensor("emb_full", [V, E], BF16, addr_space="Shared").ap()

    if debug:
        dbg_xemb = nc.dram_tensor("dbg_xemb", [E, M], FP32R, kind="ExternalOutput").ap()
        dbg_hid = nc.dram_tensor("dbg_hid", [H, M], FP32, kind="ExternalOutput").ap()
    else:
        dbg_xemb = dbg_hid = None

    with tile.TileContext(nc) as tc:
        _emit(nc, tc, locals())
    nc.compile()
    return nc


def _emit(nc, tc, t):
    xslab_d = t["xslab_d"]
    leT_d, anT_d, an3_d = t["leT_d"], t["anT_d"], t["an3_d"]
    watt_top, watt_bot, vatt, batt = t["watt_top"], t["watt_bot"], t["vatt"], t["batt"]
    wihr, whhb = t["wihr"], t["whhb"]
    b_r, b_z_neg, b_in, b_hn = t["b_r"], t["b_z_neg"], t["b_in"], t["b_hn"]
    wout, bout, maskcol, out_d = t["wout"], t["bout"], t["maskcol"], t["out_d"]
    emb_shard, emb_full = t["emb_shard"], t["emb_full"]
    dbg_xemb, dbg_hid = t["dbg_xemb"], t["dbg_hid"]

    with tc.tile_pool(name="const", bufs=1) as constp:
        # ---- DMA priority: the MLP weights, then the gather tables (they
        # gate phase A) split across all three queues; the big x^T slab last.
        watt_top_sb = constp.tile([E, ATT], BF16)
        nc.scalar.dma_start(watt_top_sb[:], watt_top[:, :])
        watt_bot_sb = constp.tile([E, ATT], BF16)
        nc.scalar.dma_start(watt_bot_sb[:], watt_bot[:, :])
        vatt_sb = constp.tile([ATT, 1], BF16)
        nc.scalar.dma_start(vatt_sb[:], vatt[:, :])
        batt_sb = constp.tile([ATT, 1], FP32)
        nc.scalar.dma_start(batt_sb[:], batt[:, :])

        gathp = tc.alloc_tile_pool(name="gath", bufs=1)
        leT = gathp.tile([128, NIDX], BF16)   # emb rows as columns
        anT = gathp.tile([128, NIDX], BF16)
        an3 = gathp.tile([128, NIDX], BF16)   # emb rows as rows, per (l, vt)
        NQ = NIDX // 4
        for q, eng in enumerate([nc.scalar, nc.sync, nc.gpsimd, nc.scalar]):
            eng.dma_start(leT[:, q * NQ : (q + 1) * NQ], leT_d[:, q * NQ : (q + 1) * NQ])
        for q, eng in enumerate([nc.sync, nc.gpsimd, nc.scalar, nc.sync]):
            eng.dma_start(anT[:, q * NQ : (q + 1) * NQ], anT_d[:, q * NQ : (q + 1) * NQ])
        for q, eng in enumerate([nc.gpsimd, nc.scalar, nc.sync, nc.gpsimd]):
            eng.dma_start(an3[:, q * NQ : (q + 1) * NQ], an3_d[:, q * NQ : (q + 1) * NQ])

        # ---- small constants ----
        wihr_sb = constp.tile([E, 3 * H], FP32R)
        nc.scalar.dma_start(wihr_sb[:], wihr[:, :])
        whhb_sb = constp.tile([H, 3 * H], BF16)
        nc.scalar.dma_start(whhb_sb[:], whhb[:, :])
        b_r_sb = constp.tile([H, 1], FP32)
        nc.scalar.dma_start(b_r_sb[:], b_r[:, :])
        b_zn_sb = constp.tile([H, 1], FP32)
        nc.scalar.dma_start(b_zn_sb[:], b_z_neg[:, :])
        b_in_sb = constp.tile([H, 1], FP32)
        nc.scalar.dma_start(b_in_sb[:], b_in[:, :])
        b_hn_sb = constp.tile([H, 1], FP32)
        nc.scalar.dma_start(b_hn_sb[:], b_hn[:, :])
        wout_sb = constp.tile([H, C], BF16)
        nc.scalar.dma_start(wout_sb[:], wout[:, :])
        bout_sb = constp.tile([1, C], FP32)
        nc.scalar.dma_start(bout_sb[:], bout[:, :])
        # mask columns per m-tile: masks_sb[p, k] = mask[k*128 + p]
        masks_sb = constp.tile([128, MT], FP32)
        nfull = (M // 128) * 128
        nc.scalar.dma_start(
            masks_sb[:, 0 : M // 128],
            maskcol[0:nfull, 0:1].rearrange("(a p) o -> p (a o)", p=128),
        )
        if M % 128:
            nc.scalar.dma_start(
                masks_sb[0 : M % 128, M // 128 : M // 128 + 1],
                maskcol[nfull:M, 0:1],
            )

        # ---- x^T slab prefetch ----
        # The DMA rings round-robin every queued transfer, so the slab would
        # steal bandwidth from the tables that gate phase A. A tiny dummy
        # read of the tables' tails stalls these engines until the tables
        # have landed, serializing tables -> slab.
        dummy = constp.tile([1, 8], BF16)
        nc.sync.dma_start(dummy[0:1, 0:2], leT[127:128, NIDX - 2 : NIDX])
        nc.gpsimd.dma_start(dummy[0:1, 2:4], anT[127:128, NIDX - 2 : NIDX])
        xslab = constp.tile([128, KT * M], BF16)
        bounds = [0, 16, 32, 48, KT]
        for i, eng in enumerate([nc.sync, nc.gpsimd, nc.sync, nc.gpsimd]):
            c0, c1 = bounds[i] * M, bounds[i + 1] * M
            eng.dma_start(xslab[:, c0:c1], xslab_d[:, c0:c1])

        # =====================================================================
        # Phase A: ontology attention on the local v-shard -> emb_shard
        # pre values are bounded (|pre| <= sum|v_att| ~ 50, tanh'd mlp), so
        # exp never overflows fp32 and the max-subtraction is skipped.
        # =====================================================================
        with (
            tc.tile_pool(name="pa_sb", bufs=2) as pa_sb,
            tc.tile_pool(name="pa_ps", bufs=2, space="PSUM") as pa_ps,
            tc.tile_pool(name="pre_ps", bufs=1, space="PSUM") as pre_ps,
        ):
            an3v = an3[:].rearrange("p (i e) -> p i e", e=E)

            # MLP: mlp_T[a, j] = tanh(watt_top.T @ leT + watt_bot.T @ anT + b)
            # pre-matmuls for (l, vt) interleave as soon as their chunk lands.
            mlp_sb = gathp.tile([ATT, NIDX], BF16)
            psp = pre_ps.tile([128, NVT * L], FP32)  # col = vt*L + l
            CH = 512
            NCH = NIDX // CH
            for ch in range(NCH + 1):
                if ch < NCH:
                    ps = pa_ps.tile([ATT, CH], FP32, tag="mlp")
                    sl = bass.ts(ch, CH)
                    nc.tensor.matmul(ps[:], watt_top_sb[:], leT[:, sl], start=True, stop=False)
                    nc.tensor.matmul(ps[:], watt_bot_sb[:], anT[:, sl], start=False, stop=True)
                    nc.scalar.activation(mlp_sb[:, sl], ps[:], AF.Tanh, bias=batt_sb[:, 0:1])
                # pre-matmuls lag one chunk so the tensor engine never
                # stalls waiting for the tanh of the chunk just issued
                if ch > 0:
                    for j in range((ch - 1) * 4, ch * 4):
                        l, vt = j // NVT, j % NVT
                        nc.tensor.matmul(
                            psp[:, vt * L + l : vt * L + l + 1],
                            mlp_sb[:, l * VP + vt * 128 : l * VP + (vt + 1) * 128],
                            vatt_sb[:],
                            start=True,
                            stop=True,
                            skip_group_check=True,
                        )

            # batched softmax over L for all vt (no max subtraction)
            att = pa_sb.tile([128, NVT * L], FP32, tag="att")
            nc.scalar.activation(att[:], psp[:], AF.Exp)
            att3 = att[:].rearrange("p (vt l) -> p vt l", l=L)
            asum = pa_sb.tile([128, NVT], FP32, tag="asum")
            nc.vector.tensor_reduce(asum[:], att3, AX.X, ALU.add)
            arec = pa_sb.tile([128, NVT], FP32, tag="arec")
            nc.vector.reciprocal(arec[:], asum[:])

            # weighted ancestor sum per vt
            for vt in range(NVT):
                acc = pa_sb.tile([128, E], FP32, tag="acc")
                nc.vector.tensor_scalar(
                    acc[:], an3v[:, 0 * NVT + vt, :], att[:, vt * L : vt * L + 1],
                    None, op0=ALU.mult,
                )
                for l in range(1, L):
                    nc.vector.scalar_tensor_tensor(
                        acc[:],
                        an3v[:, l * NVT + vt, :],
                        att[:, vt * L + l : vt * L + l + 1],
                        acc[:],
                        op0=ALU.mult,
                        op1=ALU.add,
                    )
                embt = pa_sb.tile([128, E], BF16, tag="embt")
                nc.vector.tensor_scalar(
                    embt[:], acc[:], arec[:, vt : vt + 1], None, op0=ALU.mult
                )
                rows = min(128, VS - vt * 128)
                if rows > 0:
                    nc.scalar.dma_start(
                        emb_shard[vt * 128 : vt * 128 + rows, :], embt[0:rows, :]
                    )
        gathp.release()

        # AllGather the embedding table across the 8 cores.
        nc.gpsimd.collective_compute(
            "AllGather",
            ALU.bypass,
            replica_groups=[list(range(NCORES))],
            ins=[emb_shard[:, :]],
            outs=[emb_full[:, :]],
        )

        # emb slab: [128, KT*128], block kt holds emb rows kt*128..+128.
        # Split into 8 DMAs so the first phase B matmuls start early.
        embslab = constp.tile([128, KT * 128], BF16)
        engs = [nc.sync, nc.gpsimd, nc.scalar]
        for pc in range(8):
            a0, a1 = pc * 8, min((pc + 1) * 8, KT - 1)
            engs[pc % 3].dma_start(
                embslab[:, a0 * 128 : a1 * 128].rearrange("p (a e) -> p a e", e=E),
                emb_full[a0 * 128 : a1 * 128, :].rearrange("(a p) e -> p a e", p=128),
            )
        nfull_v = (KT - 1) * 128  # 7936
        nc.gpsimd.dma_start(
            embslab[0 : V - nfull_v, nfull_v : nfull_v + 128],
            emb_full[nfull_v:V, :],
        )

        # =====================================================================
        # Phase B: x_emb^T = tanh(emb^T @ x^T)   [E, M]
        # =====================================================================
        xemb = constp.tile([E, M], FP32R)
        with tc.tile_pool(name="pb_ps", bufs=1, space="PSUM") as pb_ps:
            ps_a = pb_ps.tile([128, NH], FP32, tag="ps_a")
            ps_b = pb_ps.tile([128, NH], FP32, tag="ps_b")
            for kt in range(KT):
                kp = min(128, V - kt * 128)
                lhs = embslab[0:kp, kt * 128 : kt * 128 + E]
                st, sp = kt == 0, kt == KT - 1
                nc.tensor.matmul(
                    ps_a[:], lhs, xslab[0:kp, kt * M : kt * M + NH], start=st, stop=sp
                )
                nc.tensor.matmul(
                    ps_b[:], lhs, xslab[0:kp, kt * M + NH : kt * M + M], start=st, stop=sp
                )
            nc.scalar.activation(xemb[:, 0:NH], ps_a[:], AF.Tanh)
            nc.scalar.activation(xemb[:, NH:M], ps_b[:], AF.Tanh)
            if dbg_xemb is not None:
                nc.sync.dma_start(dbg_xemb[:, :], xemb[:])

        # =====================================================================
        # Gx precompute: input halves of all gates, biases folded.
        #   GxRZ[h, t*32+0:16]  = wih_r.T @ xemb_t + (b_ih_r + b_hh_r)
        #   GxRZ[h, t*32+16:32] = -(wih_z.T @ xemb_t + b_ih_z + b_hh_z)
        #   Gxn [h, t*16:+16]   = wih_n.T @ xemb_t + b_in
        # (z block of wihr pre-negated host-side, b_zn = -(b_ih_z+b_hh_z))
        # =====================================================================
        # gxall: per step t, 48 columns [gx_r+b_r | -(gx_z+b_z) | b_hn rep]
        gxall = constp.tile([H, T * 3 * BL], FP32)
        gxn = constp.tile([H, M], FP32)
        gx3 = gxall[:].rearrange("p (t q) -> p t q", q=3 * BL)
        nc.vector.memset(gx3[:, :, 2 * BL : 3 * BL], 0.0)
        nc.vector.tensor_scalar(
            gx3[:, :, 2 * BL : 3 * BL], gx3[:, :, 2 * BL : 3 * BL],
            b_hn_sb[:, 0:1], None, op0=ALU.add,
        )
        with tc.tile_pool(name="gx_ps", bufs=2, space="PSUM") as gx_ps:
            for half in range(2):
                xe = xemb[:, half * NH : (half + 1) * NH]
                t0, t1_ = half * (T // 2), (half + 1) * (T // 2)
                for g, bias in enumerate([b_r_sb, b_zn_sb, b_in_sb]):
                    psg = gx_ps.tile([H, NH], FP32, tag=f"gx{g % 2}")
                    nc.tensor.matmul(
                        psg[:], wihr_sb[:, g * H : (g + 1) * H], xe,
                        start=True, stop=True,
                    )
                    src = psg[:].rearrange("p (t b) -> p t b", b=BL)
                    if g == 0:
                        dst = gx3[:, t0:t1_, 0:BL]
                    elif g == 1:
                        dst = gx3[:, t0:t1_, BL : 2 * BL]
                    else:
                        dst = gxn[:, half * NH : (half + 1) * NH].rearrange(
                            "p (t b) -> p t b", b=BL
                        )
                    nc.vector.tensor_scalar(dst, src, bias[:, 0:1], None, op0=ALU.add)

        # =====================================================================
        # GRU scan; out logits computed per completed 128-token tile.
        # h kept in bf16 [H, M] (column t*BL+b).
        # =====================================================================
        hid = constp.tile([H, M], BF16)
        h0 = constp.tile([H, BL], BF16)
        nc.vector.memset(h0[:], 0.0)
        logits = constp.tile([128, MT * C], FP32)
        bb_sb = constp.tile([128, C], FP32)
        ones_sb = constp.tile([1, 128], FP32)
        nc.vector.memset(ones_sb[:], 1.0)

        QW = 3 * BL  # 48 psum columns per step: [r | zc | n]
        with (
            tc.tile_pool(name="gru_ps", bufs=2, space="PSUM") as gru_ps,
            tc.tile_pool(name="o_ps", bufs=2, space="PSUM") as o_ps,
            tc.tile_pool(name="gru_sb", bufs=3) as gru_sb,
        ):
            # broadcast b_out across partitions with a rank-1 matmul
            psbb = o_ps.tile([128, C], FP32, tag="o")
            nc.tensor.matmul(psbb[:], ones_sb[:], bout_sb[:], start=True, stop=True)
            nc.vector.tensor_copy(bb_sb[:], psbb[:])

            # prefill Gx into psum in blocks of 8 steps
            NB = 8
            NBLK = (T + NB - 1) // NB
            g_blk = [None] * NBLK
            g_blk[0] = gru_ps.tile([H, NB * QW], FP32, tag="gblk", name="gblk0")
            nc.vector.tensor_copy(g_blk[0][:], gxall[:, 0 : NB * QW])

            for st_ in range(T):
                blk, j = st_ // NB, st_ % NB
                g = g_blk[blk][:, j * QW : (j + 1) * QW]
                hprev = h0[:] if st_ == 0 else hid[:, (st_ - 1) * BL : st_ * BL]

                # hh matmuls accumulate onto the DVE-prefilled psum tile
                nc.tensor.matmul(
                    g[:, 0:BL], whhb_sb[:, 0:H], hprev,
                    start=False, stop=True, skip_group_check=True,
                )
                nc.tensor.matmul(
                    g[:, BL : 2 * BL], whhb_sb[:, H : 2 * H], hprev,
                    start=False, stop=True, skip_group_check=True,
                )
                nc.tensor.matmul(
                    g[:, 2 * BL : 3 * BL], whhb_sb[:, 2 * H : 3 * H], hprev,
                    start=False, stop=True, skip_group_check=True,
                )

                # r | zc in one sigmoid (z pre-negated => zc = 1-z)
                rz = gru_sb.tile([H, 2 * BL], BF16, tag="rz")
                nc.scalar.activation(rz[:], g[:, 0 : 2 * BL], AF.Sigmoid)

                # n = tanh(gxn + r * (ghn + b_hn))   (b_hn came via prefill)
                t1 = gru_sb.tile([H, BL], FP32, tag="t1")
                nc.vector.tensor_mul(t1[:], g[:, 2 * BL : 3 * BL], rz[:, 0:BL])
                t2 = gru_sb.tile([H, BL], FP32, tag="t2")
                nc.vector.tensor_add(t2[:], t1[:], gxn[:, st_ * BL : (st_ + 1) * BL])

                # mid-block: prefill the next block's psum
                if j == NB // 2 and blk + 1 < NBLK:
                    cols = min(NB * QW, (T - (blk + 1) * NB) * QW)
                    g_blk[blk + 1] = gru_ps.tile(
                        [H, NB * QW], FP32, tag="gblk", name=f"gblk{blk + 1}"
                    )
                    nc.vector.tensor_copy(
                        g_blk[blk + 1][:, 0:cols],
                        gxall[:, (blk + 1) * NB * QW : (blk + 1) * NB * QW + cols],
                    )

                # q = zc*h ; P = h - q run on DVE during the tanh, so only
                # two DVE ops (u, h') remain on the chain after it
                q = gru_sb.tile([H, BL], FP32, tag="q")
                nc.vector.tensor_mul(q[:], rz[:, BL : 2 * BL], hprev)
                P = gru_sb.tile([H, BL], FP32, tag="P")
                nc.vector.tensor_sub(P[:], hprev, q[:])

                n = gru_sb.tile([H, BL], BF16, tag="n")
                nc.scalar.activation(n[:], t2[:], AF.Tanh)

                # h' = zc*n + (h - zc*h)
                u = gru_sb.tile([H, BL], FP32, tag="u")
                nc.vector.tensor_mul(u[:], rz[:, BL : 2 * BL], n[:])
                nc.vector.tensor_add(hid[:, st_ * BL : (st_ + 1) * BL], u[:], P[:])

                # logits for each completed m-tile of 128 tokens
                if (st_ + 1) % 8 == 0 or st_ == T - 1:
                    k = st_ // 8
                    mw = min(128, M - k * 128)
                    pso = o_ps.tile([128, C], FP32, tag="o")
                    nc.tensor.matmul(
                        pso[0:mw, :], hid[:, k * 128 : k * 128 + mw], wout_sb[:],
                        start=True, stop=True,
                    )
                    nc.vector.tensor_add(
                        logits[0:mw, k * C : (k + 1) * C], pso[0:mw, :], bb_sb[0:mw, :]
                    )
                    if dbg_hid is not None:
                        nc.sync.dma_start(
                            dbg_hid[:, k * 128 : k * 128 + mw],
                            hid[:, k * 128 : k * 128 + mw],
                        )

        # =====================================================================
        # Softmax tail (single Exp table load)
        # =====================================================================
        # logits are bounded (|h|<=1, W_out in [0, 0.1]) so exp never
        # overflows fp32 and the max-subtraction is skipped.
        with tc.tile_pool(name="sm_sb", bufs=2) as sm_sb:
            for k in range(MT):
                mw = min(128, M - k * 128)
                lg = logits[0:mw, k * C : (k + 1) * C]
                ex = sm_sb.tile([128, C], FP32, tag="ex")
                ssum = sm_sb.tile([128, 1], FP32, tag="ssum")
                nc.scalar.activation(
                    ex[0:mw, :], lg, AF.Exp, accum_out=ssum[0:mw, 0:1]
                )
                rec = sm_sb.tile([128, 1], FP32, tag="rec")
                nc.vector.reciprocal(rec[0:mw, :], ssum[0:mw, :])
                ob = sm_sb.tile([128, C], FP32, tag="ob")
                nc.vector.tensor_scalar(
                    ob[0:mw, :], ex[0:mw, :], rec[0:mw, 0:1], masks_sb[0:mw, k : k + 1],
                    op0=ALU.mult, op1=ALU.mult,
                )
                eng = [nc.sync, nc.gpsimd, nc.scalar][k % 3]
                eng.dma_start(out_d[k * 128 : k * 128 + mw, :], ob[0:mw, :])


def _prep_inputs(inputs):
    x = np.asarray(inputs["x"], np.float32)
    mask = np.asarray(inputs["mask"], np.float32)
    W_emb = np.asarray(inputs["W_emb"], np.float32)
    W_att = np.asarray(inputs["W_attention"], np.float32)
    b_att = np.asarray(inputs["b_attention"], np.float32)
    v_att = np.asarray(inputs["v_attention"], np.float32)
    w_ih = np.asarray(inputs["gru_w_ih"], np.float32)
    w_hh = np.asarray(inputs["gru_w_hh"], np.float32)
    b_ih = np.asarray(inputs["gru_b_ih"], np.float32)
    b_hh = np.asarray(inputs["gru_b_hh"], np.float32)
    W_out = np.asarray(inputs["W_output"], np.float32)
    b_out = np.asarray(inputs["b_output"], np.float32)
    leaves = np.asarray(inputs["leaves"])
    ancestors = np.asarray(inputs["ancestors"])

    # wih^T [E, 3H] with z block negated (fp32, used via fp32r)
    wih_t = np.ascontiguousarray(w_ih.T).copy()
    wih_t[:, H : 2 * H] = -wih_t[:, H : 2 * H]
    # whh^T [H, 3H] with z block negated (bf16)
    whh_t = np.ascontiguousarray(w_hh.T).copy()
    whh_t[:, H : 2 * H] = -whh_t[:, H : 2 * H]

    shared = {
        "watt_top": np.ascontiguousarray(W_att[:E, :].astype(ml_dtypes.bfloat16)),
        "watt_bot": np.ascontiguousarray(W_att[E:, :].astype(ml_dtypes.bfloat16)),
        "vatt": np.ascontiguousarray(v_att.reshape(ATT, 1).astype(ml_dtypes.bfloat16)),
        "batt": np.ascontiguousarray(b_att.reshape(ATT, 1)),
        "wihr": np.ascontiguousarray(wih_t),
        "whhb": np.ascontiguousarray(whh_t.astype(ml_dtypes.bfloat16)),
        "b_r": np.ascontiguousarray((b_ih[0:H] + b_hh[0:H]).reshape(H, 1)),
        "b_z_neg": np.ascontiguousarray(-(b_ih[H : 2 * H] + b_hh[H : 2 * H]).reshape(H, 1)),
        "b_in": np.ascontiguousarray(b_ih[2 * H : 3 * H].reshape(H, 1)),
        "b_hn": np.ascontiguousarray(b_hh[2 * H : 3 * H].reshape(H, 1)),
        "wout": np.ascontiguousarray(W_out.astype(ml_dtypes.bfloat16)),
        "bout": np.ascontiguousarray(b_out.reshape(1, C)),
    }

    W_bf = W_emb.astype(ml_dtypes.bfloat16)
    in_maps = []
    for c in range(NCORES):
        m = dict(shared)
        xc = x[:, c * BL : (c + 1) * BL, :].reshape(M, V)
        xcT = np.zeros((KT * 128, M), ml_dtypes.bfloat16)
        xcT[:V, :] = np.ascontiguousarray(xc.T).astype(ml_dtypes.bfloat16)
        # slab[p, kt*M + m] = x^T[kt*128 + p, m]
        m["xslab_d"] = np.ascontiguousarray(
            xcT.reshape(KT, 128, M).transpose(1, 0, 2).reshape(128, KT * M)
        )
        m["maskcol"] = np.ascontiguousarray(
            mask[:, c * BL : (c + 1) * BL].reshape(M, 1)
        )
        lv = leaves[c * VS : (c + 1) * VS, :]
        av = ancestors[c * VS : (c + 1) * VS, :]
        le_pad = np.zeros((L, VP), np.int64)   # [l, v] l-major, v padded
        le_pad[:, :VS] = lv.T
        an_pad = np.zeros((L, VP), np.int64)
        an_pad[:, :VS] = av.T
        # pre-gathered, pre-transposed embedding rows (static index prep)
        le_rows = W_bf[le_pad.reshape(-1), :]          # [NIDX, E]
        an_rows = W_bf[an_pad.reshape(-1), :]          # [NIDX, E]
        m["leT_d"] = np.ascontiguousarray(le_rows.T)   # [E, NIDX]
        m["anT_d"] = np.ascontiguousarray(an_rows.T)   # [E, NIDX]
        # an3[p, (l*NVT+vt)*E + e] = an_rows[l*VP + vt*128 + p, e]
        m["an3_d"] = np.ascontiguousarray(
            an_rows.reshape(L, NVT, 128, E).transpose(2, 0, 1, 3).reshape(128, NIDX)
        )
        in_maps.append(m)
    return in_maps


def kernel(**inputs):
    if "nc" not in _CACHE:
        _CACHE["nc"] = _build_nc()
    nc = _CACHE["nc"]
    in_maps = _prep_inputs(inputs)
    res = run_bass_kernel_spmd(nc, in_maps, list(range(NCORES)))
    out = np.empty((T, B, C), np.float32)
    for c in range(NCORES):
        out[:, c * BL : (c + 1) * BL, :] = res.results[c]["out"].reshape(T, BL, C)
    return out
